# revision 11
# baseline (speedup 1.0000x reference)
"""Sparse 3x3x3 submanifold-conv block (gather -> per-offset GEMM -> scatter-add
-> BatchNorm -> ReLU) on 8 TRN2 NeuronCores.

Sharding: data-parallel over output voxels; core c owns output rows
[c*12500, (c+1)*12500). x and W replicated; BN stats all-reduced on device.

HW-verified primitives: dma_gather (int16 idx, 16-wrap, windowed src view) and
dma_scatter_add (int16 idx; duplicate targets within one call RACE -> must be
unique per call; calls to the same tensor are serialized by Tile).

Layout: tiles of 128 pairs arranged as (bucket b, round r, offset k).
- bucket = src_row // 32768 (int16 gather window)
- each (k, b) has T_kb tiles (equalized across cores for SPMD); round r of
  bucket b holds tile (k, b, r) for every k with T_kb > r.
- one dma_gather per (b, r): single src window, contiguous tiles.
- one dma_scatter_add per (b, r): pair targets within a round are unique by
  construction: a voxel's duplicate contributions are dealt across rounds and
  across an occurrence-parity split of the accumulator (rows tgt + 12544*par).
- pad slots gather row 0 of the window and scatter into a trash row (12500),
  adding garbage only there.
"""

import numpy as np

N = 100000
C = 64
K3 = 27
M = 40000
NCORES = 8
VS = N // NCORES            # 12500
VPAD = 12544                # half-accumulator rows (trash rows 12500..12543)
TRASH = VS
EPS = 1e-5
WIN = 32768
BASES = np.array([0, WIN, 2 * WIN, 3 * WIN])
WINS = [WIN, WIN, WIN, N - 3 * WIN]

_cache = {}


def _build(cfg, rep=1, tiny_gather=False, tiny_scatter=False):
    import concourse.bacc as bacc
    import concourse.tile as tile
    import concourse.mybir as mybir
    from concourse.masks import make_identity
    import contextlib

    f32 = mybir.dt.float32
    i16 = mybir.dt.int16

    calls = cfg["calls"]          # list of (bucket, n_tiles, tile_offset)
    ktile = cfg["ktile"]
    T_TOT = cfg["T_TOT"]
    NT_OUT = VPAD // 128          # 98 tiles per accumulator half
    LAST_REAL = VS - (NT_OUT - 1) * 128   # 84
    AROWS = 2 * VPAD              # 25088
    MAXNT = max(nt for _, nt, _ in calls)

    nc = bacc.Bacc("TRN2", target_bir_lowering=False, debug=False)
    x_d = nc.dram_tensor("x", [N, C], f32, kind="ExternalInput")
    gx_d = nc.dram_tensor("gx", [128, T_TOT * 8], i16, kind="ExternalInput")
    sidx_d = nc.dram_tensor("sidx", [128, T_TOT * 8], i16, kind="ExternalInput")
    w_d = nc.dram_tensor("w", [2 * C, K3 * C], f32, kind="ExternalInput")
    gb_d = nc.dram_tensor("gb", [1, 2 * C], f32, kind="ExternalInput")
    op_d = nc.dram_tensor("opart", [128, 1], f32, kind="ExternalInput")
    out_d = nc.dram_tensor("out", [VS, C], f32, kind="ExternalOutput")

    with tile.TileContext(nc) as tc:
        with contextlib.ExitStack() as ctx:
            const = ctx.enter_context(tc.tile_pool(name="const", bufs=1))
            featp = ctx.enter_context(tc.tile_pool(name="feat", bufs=3))
            ftp = ctx.enter_context(tc.tile_pool(name="featT", bufs=3))
            ctp = ctx.enter_context(tc.tile_pool(name="contrib", bufs=3))
            psT = ctx.enter_context(tc.tile_pool(name="psT", bufs=2, space="PSUM"))
            psM = ctx.enter_context(tc.tile_pool(name="psM", bufs=4, space="PSUM"))
            psS = ctx.enter_context(tc.tile_pool(name="psS", bufs=1, space="PSUM"))
            dram = ctx.enter_context(tc.tile_pool(name="dram", bufs=1, space="DRAM"))
            bnp = ctx.enter_context(tc.tile_pool(name="bn", bufs=6))
            outp = ctx.enter_context(tc.tile_pool(name="outp", bufs=4))

            gx_sb = const.tile([128, T_TOT * 8], i16)
            nc.sync.dma_start(out=gx_sb[:], in_=gx_d[:])
            sidx_sb = const.tile([128, T_TOT * 8], i16)
            nc.sync.dma_start(out=sidx_sb[:], in_=sidx_d[:])
            w_sb = const.tile([2 * C, K3 * C], f32)
            nc.sync.dma_start(out=w_sb[:], in_=w_d[:])
            gb_sb = const.tile([1, 2 * C], f32)
            nc.sync.dma_start(out=gb_sb[:], in_=gb_d[:])
            id_sb = const.tile([128, 128], f32)
            make_identity(nc, id_sb[:])
            ones_full = const.tile([128, 1], f32)
            nc.vector.memset(ones_full[:], 1.0)
            ones_part = const.tile([128, 1], f32)
            nc.sync.dma_start(out=ones_part[:], in_=op_d[:])
            ones_row = const.tile([1, 128], f32)
            nc.vector.memset(ones_row[:], 1.0)

            accs = [dram.tile([AROWS, C], f32, name=f"acc{i}", tag=f"acc{i}")
                    for i in range(2)]
            zsb = const.tile([128, 784], f32)
            nc.vector.memset(zsb[:], 0.0)
            for acc in accs:
                for r in range(16):
                    nc.sync.dma_start(
                        out=acc[r * 1568:(r + 1) * 1568, :], in_=zsb[:])

            # ---- main pipeline: one gather + one scatter per (bucket, round) ----
            for ci, (b, nt, t0) in enumerate(calls * rep):
                ni = nt * 128
                feat = featp.tile([128, MAXNT, C], f32, tag="feat")
                gni = 128 if tiny_gather else ni
                nc.gpsimd.dma_gather(
                    feat[:, 0:gni // 128, :],
                    x_d[int(BASES[b]):int(BASES[b]) + WINS[b], :],
                    gx_sb[:, t0 * 8:t0 * 8 + gni // 16],
                    gni, gni, C, single_packet=False,
                )
                featT = ftp.tile([128, ((MAXNT + 1) // 2) * 128], f32, tag="featT")
                for j in range((nt + 1) // 2):
                    w2 = min(2, nt - j * 2)
                    pt = psT.tile([128, 128], f32, tag="pt")
                    nc.tensor.transpose(
                        out=pt[0:w2 * C, :],
                        in_=feat[:, j * 2:j * 2 + w2, :],
                        identity=id_sb[:],
                    )
                    nc.scalar.copy(
                        out=featT[0:w2 * C, j * 128:(j + 1) * 128],
                        in_=pt[0:w2 * C, :])
                contrib = ctp.tile([128, MAXNT, C], f32, tag="contrib")
                for tr in range(nt):
                    k = ktile[t0 + tr]
                    j, half = tr // 2, tr % 2
                    pm = psM.tile([128, C], f32, tag="pm")
                    nc.tensor.matmul(
                        out=pm[:],
                        lhsT=featT[half * C:(half + 1) * C,
                                   j * 128:(j + 1) * 128],
                        rhs=w_sb[half * C:(half + 1) * C, k * C:(k + 1) * C],
                        start=True, stop=True,
                    )
                    nc.vector.tensor_copy(out=contrib[:, tr, :], in_=pm[:])
                acc = accs[ci % 2]
                sni = 128 if tiny_scatter else ni
                nc.gpsimd.dma_scatter_add(
                    acc[:],
                    contrib[:, 0:sni // 128, :],
                    sidx_sb[:, t0 * 8:t0 * 8 + sni // 16],
                    sni, sni, C, single_packet=False,
                )

            # ---- BN stats over the 4 accumulator halves ----
            sums_ps = psS.tile([1, 128], f32)
            acc_sum = dram.tile([VPAD, C], f32)
            for i in range(NT_OUT):
                parts = []
                for pi, (ai, ao) in enumerate(((0, 0), (0, VPAD), (1, 0), (1, VPAD))):
                    t = bnp.tile([128, C], f32, name=f"bnl{pi}", tag=f"bnl{pi}")
                    nc.sync.dma_start(
                        out=t[:],
                        in_=accs[ai][ao + i * 128:ao + (i + 1) * 128, :])
                    parts.append(t)
                s01 = bnp.tile([128, C], f32, tag="s01")
                nc.vector.tensor_add(out=s01[:], in0=parts[0][:], in1=parts[1][:])
                s23 = bnp.tile([128, C], f32, tag="s23")
                nc.vector.tensor_add(out=s23[:], in0=parts[2][:], in1=parts[3][:])
                s = bnp.tile([128, C], f32, tag="s")
                nc.vector.tensor_add(out=s[:], in0=s01[:], in1=s23[:])
                nc.sync.dma_start(out=acc_sum[i * 128:(i + 1) * 128, :], in_=s[:])
                sq = bnp.tile([128, C], f32, tag="sq")
                nc.scalar.square(sq[:], s[:])
                ones = ones_full if i < NT_OUT - 1 else ones_part
                nc.tensor.matmul(out=sums_ps[:, 0:C], lhsT=ones[:], rhs=s[:],
                                 start=(i == 0), stop=(i == NT_OUT - 1))
                nc.tensor.matmul(out=sums_ps[:, C:2 * C], lhsT=ones[:], rhs=sq[:],
                                 start=(i == 0), stop=(i == NT_OUT - 1))

            stat_sb = const.tile([1, 128], f32)
            nc.vector.tensor_copy(out=stat_sb[:], in_=sums_ps[:])
            cc_in = dram.tile([1, 128], f32)
            cc_out = dram.tile([1, 128], f32)
            nc.gpsimd.dma_start(out=cc_in[:], in_=stat_sb[:])
            nc.gpsimd.collective_compute(
                "AllReduce", mybir.AluOpType.add,
                replica_groups=[list(range(NCORES))],
                ins=[cc_in.opt()], outs=[cc_out.opt()],
            )
            gstat = const.tile([1, 128], f32)
            nc.gpsimd.dma_start(out=gstat[:], in_=cc_out[:])

            mean = const.tile([1, C], f32)
            nc.vector.tensor_scalar_mul(mean[:], gstat[:, 0:C], 1.0 / N)
            ex2 = const.tile([1, C], f32)
            nc.vector.tensor_scalar_mul(ex2[:], gstat[:, C:2 * C], 1.0 / N)
            m2 = const.tile([1, C], f32)
            nc.vector.tensor_tensor(out=m2[:], in0=mean[:], in1=mean[:],
                                    op=mybir.AluOpType.mult)
            var = const.tile([1, C], f32)
            nc.vector.tensor_tensor(out=var[:], in0=ex2[:], in1=m2[:],
                                    op=mybir.AluOpType.subtract)
            vpe = const.tile([1, C], f32)
            nc.vector.tensor_scalar_add(vpe[:], var[:], EPS)
            std = const.tile([1, C], f32)
            nc.scalar.sqrt(std[:], vpe[:])
            rstd = const.tile([1, C], f32)
            nc.vector.reciprocal(rstd[:], std[:])
            scale = const.tile([1, C], f32)
            nc.vector.tensor_tensor(out=scale[:], in0=rstd[:], in1=gb_sb[0:1, 0:C],
                                    op=mybir.AluOpType.mult)
            msc = const.tile([1, C], f32)
            nc.vector.tensor_tensor(out=msc[:], in0=mean[:], in1=scale[:],
                                    op=mybir.AluOpType.mult)
            bias = const.tile([1, C], f32)
            nc.vector.tensor_tensor(out=bias[:], in0=gb_sb[0:1, C:2 * C],
                                    in1=msc[:], op=mybir.AluOpType.subtract)
            sb_ps = psS.tile([128, 2 * C], f32)
            nc.tensor.matmul(out=sb_ps[:, 0:C], lhsT=ones_row[:], rhs=scale[:],
                             start=True, stop=True)
            nc.tensor.matmul(out=sb_ps[:, C:2 * C], lhsT=ones_row[:], rhs=bias[:],
                             start=True, stop=True)
            sc_bc = const.tile([128, C], f32)
            nc.vector.tensor_copy(out=sc_bc[:], in_=sb_ps[:, 0:C])
            bi_bc = const.tile([128, C], f32)
            nc.vector.tensor_copy(out=bi_bc[:], in_=sb_ps[:, C:2 * C])

            for i in range(NT_OUT):
                sl = outp.tile([128, C], f32, tag="sl")
                nc.sync.dma_start(out=sl[:], in_=acc_sum[i * 128:(i + 1) * 128, :])
                y = outp.tile([128, C], f32, tag="y")
                nc.vector.tensor_tensor(out=y[:], in0=sl[:], in1=sc_bc[:],
                                        op=mybir.AluOpType.mult)
                y2 = outp.tile([128, C], f32, tag="y2")
                nc.vector.tensor_tensor(out=y2[:], in0=y[:], in1=bi_bc[:],
                                        op=mybir.AluOpType.add)
                nc.vector.tensor_scalar_max(y2[:], y2[:], 0.0)
                rows = 128 if i < NT_OUT - 1 else LAST_REAL
                nc.sync.dma_start(out=out_d[i * 128:i * 128 + rows, :],
                                  in_=y2[0:rows, :])

    nc.compile()
    return nc


def _assign(src, tgt, kk, T_kb, rounds_b, tile_base):
    """Assign one core's pairs to tile slots.

    Returns per-pair packed value slot*2+parity, or None if infeasible.
    Within a round (b, r), final scatter targets tgt + VPAD*parity are
    unique by construction.
    """
    npairs = len(src)
    out = np.full(npairs, -1, dtype=np.int64)
    b_arr = np.minimum(src // WIN, 3)
    fill = np.zeros_like(tile_base)
    for b in range(4):
        sel = np.where(b_arr == b)[0]
        if len(sel) == 0:
            continue
        order = sel[np.argsort(tgt[sel], kind="stable")]
        i = 0
        Rb = int(rounds_b[b])
        tgt_o = tgt[order]
        while i < len(order):
            j = i
            t = tgt_o[i]
            while j < len(order) and tgt_o[j] == t:
                j += 1
            group = order[i:j]
            used = (set(), set())
            for gi, pi in enumerate(group):
                par = gi & 1
                k = kk[pi]
                placed = False
                for p_try in (par, 1 - par):
                    cand = [r for r in range(min(Rb, int(T_kb[k, b])))
                            if r not in used[p_try] and fill[k, b, r] < 128]
                    if cand:
                        r = min(cand, key=lambda rr: fill[k, b, rr])
                        out[pi] = (tile_base[k, b, r] * 128
                                   + fill[k, b, r]) * 2 + p_try
                        fill[k, b, r] += 1
                        used[p_try].add(r)
                        placed = True
                        break
                if not placed:
                    return None
            i = j
    return out


def _prep(x, W, gamma, beta, in_idx, out_idx):
    k_arr = np.repeat(np.arange(K3, dtype=np.int64), M)
    in_flat = in_idx.reshape(-1).astype(np.int64)
    out_flat = out_idx.reshape(-1).astype(np.int64)
    owner = out_flat // VS

    cores = []
    cnt = np.zeros((NCORES, K3, 4), dtype=np.int64)
    for c in range(NCORES):
        sel = owner == c
        src, tgt, kk = in_flat[sel], out_flat[sel] - c * VS, k_arr[sel]
        b = np.minimum(src // WIN, 3)
        for k in range(K3):
            ks = kk == k
            cnt[c, k] += np.bincount(b[ks], minlength=4)
        cores.append((src, tgt, kk))

    slack = 0
    while True:
        T_kb = -(-cnt.max(axis=0) // 128) + slack
        T_kb = np.maximum(T_kb, 1)
        rounds_b = T_kb.max(axis=0)
        maxR = int(rounds_b.max())
        tile_base = np.full((K3, 4, maxR), -1, dtype=np.int64)
        calls = []
        ktile = []
        t = 0
        for b in range(4):
            for r in range(int(rounds_b[b])):
                ks = [k for k in range(K3) if T_kb[k, b] > r]
                if not ks:
                    continue
                calls.append((b, len(ks), t))
                for k in ks:
                    tile_base[k, b, r] = t
                    ktile.append(k)
                    t += 1
        T_TOT = t
        slots = [_assign(src, tgt, kk, T_kb, rounds_b, tile_base)
                 for (src, tgt, kk) in cores]
        if all(s is not None for s in slots):
            break
        slack += 1
        assert slack < 6, "round assignment failed repeatedly"

    in_maps = []
    for c in range(NCORES):
        src, tgt, kk = cores[c]
        sl = slots[c]
        par = sl % 2
        pos = sl // 2
        gx = np.zeros(T_TOT * 128, dtype=np.int16)
        sx = np.full(T_TOT * 128, TRASH, dtype=np.int16)
        gx[pos] = (src - BASES[np.minimum(src // WIN, 3)]).astype(np.int16)
        sx[pos] = (tgt + VPAD * par).astype(np.int16)
        wrap = lambda a: np.tile(a.reshape(-1, 16).T.copy(), (8, 1))
        in_maps.append({
            "x": np.ascontiguousarray(x, dtype=np.float32),
            "gx": wrap(gx),
            "sidx": wrap(sx),
            "w": np.ascontiguousarray(np.concatenate(
                [W.transpose(1, 0, 2).reshape(C, K3 * C)] * 2, axis=0),
                dtype=np.float32),
            "gb": np.concatenate([gamma, beta]).astype(np.float32).reshape(1, 2 * C),
            "opart": (np.arange(128) < (VS - (VPAD // 128 - 1) * 128)
                      ).astype(np.float32).reshape(128, 1),
        })
    cfg = {"calls": calls, "ktile": ktile, "T_TOT": T_TOT}
    return in_maps, cfg


def kernel(x, W, gamma, beta, in_idx, out_idx):
    from concourse.bass_utils import run_bass_kernel_spmd

    x = np.asarray(x); W = np.asarray(W)
    gamma = np.asarray(gamma); beta = np.asarray(beta)
    in_idx = np.asarray(in_idx); out_idx = np.asarray(out_idx)

    in_maps, cfg = _prep(x, W, gamma, beta, in_idx, out_idx)
    key = (cfg["T_TOT"], tuple(cfg["ktile"]),
           tuple((b, n, t) for b, n, t in cfg["calls"]))
    if key not in _cache:
        _cache[key] = _build(cfg)
    nc = _cache[key]
    res = run_bass_kernel_spmd(nc, in_maps, core_ids=list(range(NCORES)))
    return np.concatenate([res.results[c]["out"] for c in range(NCORES)], axis=0)


# revision 13
# speedup vs baseline: 1.1756x; 1.1756x over previous
"""Sparse 3x3x3 submanifold-conv block (gather -> per-offset GEMM -> scatter-add
-> BatchNorm -> ReLU) on 8 TRN2 NeuronCores.

Sharding: data-parallel over output voxels; core c owns output rows
[c*12500, (c+1)*12500). x and W replicated; BN stats all-reduced on device.

HW-verified primitives: dma_gather (int16 idx, 16-wrap, windowed src view) and
dma_scatter_add (int16 idx; duplicate targets within one call RACE -> must be
unique per call; calls to the same tensor are serialized by Tile).

Layout: tiles of 128 pairs arranged as (bucket b, round r, offset k).
- bucket = src_row // 32768 (int16 gather window)
- each (k, b) has T_kb tiles (equalized across cores for SPMD); round r of
  bucket b holds tile (k, b, r) for every k with T_kb > r.
- one dma_gather per (b, r): single src window, contiguous tiles.
- one dma_scatter_add per (b, r): pair targets within a round are unique by
  construction: a voxel's duplicate contributions are dealt across rounds and
  across an occurrence-parity split of the accumulator (rows tgt + 12544*par).
- pad slots gather row 0 of the window and scatter into a trash row (12500),
  adding garbage only there.
"""

import numpy as np

N = 100000
C = 64
K3 = 27
M = 40000
NCORES = 8
VS = N // NCORES            # 12500
VPAD = 12544                # half-accumulator rows (trash rows 12500..12543)
TRASH = VS
EPS = 1e-5
WIN = 32768
BASES = np.array([0, WIN, 2 * WIN, 3 * WIN])
WINS = [WIN, WIN, WIN, N - 3 * WIN]

_cache = {}


def _build(cfg, rep=1, tiny_gather=False, tiny_scatter=False):
    import concourse.bacc as bacc
    import concourse.tile as tile
    import concourse.mybir as mybir
    from concourse.masks import make_identity
    import contextlib

    f32 = mybir.dt.float32
    i16 = mybir.dt.int16

    calls = cfg["calls"]          # list of (bucket, n_tiles, tile_offset)
    ktile = cfg["ktile"]
    T_TOT = cfg["T_TOT"]
    NT_OUT = VPAD // 128          # 98 tiles per accumulator half
    LAST_REAL = VS - (NT_OUT - 1) * 128   # 84
    AROWS = 2 * VPAD              # 25088
    MAXNT = max(nt for _, nt, _ in calls)

    nc = bacc.Bacc("TRN2", target_bir_lowering=False, debug=False)
    x_d = nc.dram_tensor("x", [N, C], f32, kind="ExternalInput")
    gx_d = nc.dram_tensor("gx", [128, T_TOT * 8], i16, kind="ExternalInput")
    sidx_d = nc.dram_tensor("sidx", [128, T_TOT * 8], i16, kind="ExternalInput")
    w_d = nc.dram_tensor("w", [2 * C, K3 * C], f32, kind="ExternalInput")
    gb_d = nc.dram_tensor("gb", [1, 2 * C], f32, kind="ExternalInput")
    op_d = nc.dram_tensor("opart", [128, 1], f32, kind="ExternalInput")
    out_d = nc.dram_tensor("out", [VS, C], f32, kind="ExternalOutput")

    with tile.TileContext(nc) as tc:
        with contextlib.ExitStack() as ctx:
            const = ctx.enter_context(tc.tile_pool(name="const", bufs=1))
            featp = ctx.enter_context(tc.tile_pool(name="feat", bufs=3))
            ftp = ctx.enter_context(tc.tile_pool(name="featT", bufs=3))
            ctp = ctx.enter_context(tc.tile_pool(name="contrib", bufs=3))
            psT = ctx.enter_context(tc.tile_pool(name="psT", bufs=2, space="PSUM"))
            psM = ctx.enter_context(tc.tile_pool(name="psM", bufs=4, space="PSUM"))
            psS = ctx.enter_context(tc.tile_pool(name="psS", bufs=1, space="PSUM"))
            dram = ctx.enter_context(tc.tile_pool(name="dram", bufs=1, space="DRAM"))
            bnp = ctx.enter_context(tc.tile_pool(name="bn", bufs=6))
            outp = ctx.enter_context(tc.tile_pool(name="outp", bufs=4))

            gx_sb = const.tile([128, T_TOT * 8], i16)
            nc.sync.dma_start(out=gx_sb[:], in_=gx_d[:])
            sidx_sb = const.tile([128, T_TOT * 8], i16)
            nc.sync.dma_start(out=sidx_sb[:], in_=sidx_d[:])
            w_sb = const.tile([2 * C, K3 * C], f32)
            nc.sync.dma_start(out=w_sb[:], in_=w_d[:])
            gb_sb = const.tile([1, 2 * C], f32)
            nc.sync.dma_start(out=gb_sb[:], in_=gb_d[:])
            id_sb = const.tile([128, 128], f32)
            make_identity(nc, id_sb[:])
            ones_full = const.tile([128, 1], f32)
            nc.vector.memset(ones_full[:], 1.0)
            ones_part = const.tile([128, 1], f32)
            nc.sync.dma_start(out=ones_part[:], in_=op_d[:])
            ones_row = const.tile([1, 128], f32)
            nc.vector.memset(ones_row[:], 1.0)

            # SBUF accumulators: voxel v -> partition v%128, slot v//128;
            # group g = slot>>1 is the free column; slot parity routes own/peer.
            NSLOT = VPAD // 128            # 98
            NG = NSLOT // 2                # 49
            sb_accs = []
            for i in range(2):
                own = const.tile([128, NG * C], f32, name=f"own{i}")
                nc.vector.memset(own[:], 0.0)
                peer = const.tile([128, NG * C], f32, name=f"peer{i}")
                nc.vector.memset(peer[:], 0.0)
                sb_accs.append((own, peer))

            # ---- main pipeline: one gather + one scatter per (bucket, round) ----
            for ci, (b, nt, t0) in enumerate(calls * rep):
                ni = nt * 128
                feat = featp.tile([128, MAXNT, C], f32, tag="feat")
                gni = 128 if tiny_gather else ni
                nc.gpsimd.dma_gather(
                    feat[:, 0:gni // 128, :],
                    x_d[int(BASES[b]):int(BASES[b]) + WINS[b], :],
                    gx_sb[:, t0 * 8:t0 * 8 + gni // 16],
                    gni, gni, C, single_packet=False,
                )
                featT = ftp.tile([128, ((MAXNT + 1) // 2) * 128], f32, tag="featT")
                for j in range((nt + 1) // 2):
                    w2 = min(2, nt - j * 2)
                    pt = psT.tile([128, 128], f32, tag="pt")
                    nc.tensor.transpose(
                        out=pt[0:w2 * C, :],
                        in_=feat[:, j * 2:j * 2 + w2, :],
                        identity=id_sb[:],
                    )
                    nc.scalar.copy(
                        out=featT[0:w2 * C, j * 128:(j + 1) * 128],
                        in_=pt[0:w2 * C, :])
                contrib = ctp.tile([128, MAXNT, C], f32, tag="contrib")
                for tr in range(nt):
                    k = ktile[t0 + tr]
                    j, half = tr // 2, tr % 2
                    pm = psM.tile([128, C], f32, tag="pm")
                    nc.tensor.matmul(
                        out=pm[:],
                        lhsT=featT[half * C:(half + 1) * C,
                                   j * 128:(j + 1) * 128],
                        rhs=w_sb[half * C:(half + 1) * C, k * C:(k + 1) * C],
                        start=True, stop=True,
                    )
                    nc.vector.tensor_copy(out=contrib[:, tr, :], in_=pm[:])
                own, peer = sb_accs[ci % 2]
                sni = 128 if tiny_scatter else ni
                nc.gpsimd.dma_scatter_add(
                    own[:],
                    contrib[:, 0:sni // 128, :],
                    sidx_sb[:, t0 * 8:t0 * 8 + sni // 16],
                    sni, sni, C, single_packet=False,
                    sbuf_tokens_per_rank=128, parity_reg=0,
                    out_ap_other=peer[:],
                )

            # ---- BN stats straight from the SBUF accumulators ----
            def slot_ap(acc_i, j):
                own, peer = sb_accs[acc_i]
                t = peer if (j & 1) else own
                g = j >> 1
                return t[:, g * C:(g + 1) * C]

            sums_ps = psS.tile([1, 128], f32)
            s_tiles = []
            for j in range(NSLOT):
                s = bnp.tile([128, C], f32, tag="s")
                nc.vector.tensor_add(out=s[:], in0=slot_ap(0, j), in1=slot_ap(1, j))
                sq = bnp.tile([128, C], f32, tag="sq")
                nc.scalar.square(sq[:], s[:])
                ones = ones_full if j < NSLOT - 1 else ones_part
                nc.tensor.matmul(out=sums_ps[:, 0:C], lhsT=ones[:], rhs=s[:],
                                 start=(j == 0), stop=(j == NSLOT - 1))
                nc.tensor.matmul(out=sums_ps[:, C:2 * C], lhsT=ones[:], rhs=sq[:],
                                 start=(j == 0), stop=(j == NSLOT - 1))

            stat_sb = const.tile([1, 128], f32)
            nc.vector.tensor_copy(out=stat_sb[:], in_=sums_ps[:])
            cc_in = dram.tile([1, 128], f32)
            cc_out = dram.tile([1, 128], f32)
            nc.gpsimd.dma_start(out=cc_in[:], in_=stat_sb[:])
            nc.gpsimd.collective_compute(
                "AllReduce", mybir.AluOpType.add,
                replica_groups=[list(range(NCORES))],
                ins=[cc_in.opt()], outs=[cc_out.opt()],
            )
            gstat = const.tile([1, 128], f32)
            nc.gpsimd.dma_start(out=gstat[:], in_=cc_out[:])

            mean = const.tile([1, C], f32)
            nc.vector.tensor_scalar_mul(mean[:], gstat[:, 0:C], 1.0 / N)
            ex2 = const.tile([1, C], f32)
            nc.vector.tensor_scalar_mul(ex2[:], gstat[:, C:2 * C], 1.0 / N)
            m2 = const.tile([1, C], f32)
            nc.vector.tensor_tensor(out=m2[:], in0=mean[:], in1=mean[:],
                                    op=mybir.AluOpType.mult)
            var = const.tile([1, C], f32)
            nc.vector.tensor_tensor(out=var[:], in0=ex2[:], in1=m2[:],
                                    op=mybir.AluOpType.subtract)
            vpe = const.tile([1, C], f32)
            nc.vector.tensor_scalar_add(vpe[:], var[:], EPS)
            std = const.tile([1, C], f32)
            nc.scalar.sqrt(std[:], vpe[:])
            rstd = const.tile([1, C], f32)
            nc.vector.reciprocal(rstd[:], std[:])
            scale = const.tile([1, C], f32)
            nc.vector.tensor_tensor(out=scale[:], in0=rstd[:], in1=gb_sb[0:1, 0:C],
                                    op=mybir.AluOpType.mult)
            msc = const.tile([1, C], f32)
            nc.vector.tensor_tensor(out=msc[:], in0=mean[:], in1=scale[:],
                                    op=mybir.AluOpType.mult)
            bias = const.tile([1, C], f32)
            nc.vector.tensor_tensor(out=bias[:], in0=gb_sb[0:1, C:2 * C],
                                    in1=msc[:], op=mybir.AluOpType.subtract)
            sb_ps = psS.tile([128, 2 * C], f32)
            nc.tensor.matmul(out=sb_ps[:, 0:C], lhsT=ones_row[:], rhs=scale[:],
                             start=True, stop=True)
            nc.tensor.matmul(out=sb_ps[:, C:2 * C], lhsT=ones_row[:], rhs=bias[:],
                             start=True, stop=True)
            sc_bc = const.tile([128, C], f32)
            nc.vector.tensor_copy(out=sc_bc[:], in_=sb_ps[:, 0:C])
            bi_bc = const.tile([128, C], f32)
            nc.vector.tensor_copy(out=bi_bc[:], in_=sb_ps[:, C:2 * C])

            for i in range(NT_OUT):
                sl = outp.tile([128, C], f32, tag="sl")
                nc.vector.tensor_add(out=sl[:], in0=slot_ap(0, i), in1=slot_ap(1, i))
                y = outp.tile([128, C], f32, tag="y")
                nc.vector.tensor_tensor(out=y[:], in0=sl[:], in1=sc_bc[:],
                                        op=mybir.AluOpType.mult)
                y2 = outp.tile([128, C], f32, tag="y2")
                nc.vector.tensor_tensor(out=y2[:], in0=y[:], in1=bi_bc[:],
                                        op=mybir.AluOpType.add)
                nc.vector.tensor_scalar_max(y2[:], y2[:], 0.0)
                rows = 128 if i < NT_OUT - 1 else LAST_REAL
                nc.sync.dma_start(out=out_d[i * 128:i * 128 + rows, :],
                                  in_=y2[0:rows, :])

    nc.compile()
    return nc


def _assign(src, tgt, kk, T_kb, rounds_b, tile_base, SPILL_K=8):
    """Assign one core's pairs to tile slots.

    Constraint: within a round (b, r) a voxel appears at most once (SBUF
    CCE scatter-add races on duplicate targets within one call). A pair of
    offset k may use rounds r < T_kb[k, b], or any round if k < SPILL_K.
    Returns slot index per pair, or None if infeasible.
    """
    npairs = len(src)
    out = np.full(npairs, -1, dtype=np.int64)
    b_arr = np.minimum(src // WIN, 3)
    fill = np.zeros_like(tile_base)
    for b in range(4):
        sel = np.where(b_arr == b)[0]
        if len(sel) == 0:
            continue
        order = sel[np.argsort(tgt[sel], kind="stable")]
        i = 0
        Rb = int(rounds_b[b])
        tgt_o = tgt[order]
        while i < len(order):
            j = i
            t = tgt_o[i]
            while j < len(order) and tgt_o[j] == t:
                j += 1
            group = order[i:j]
            used = set()
            ok = True
            for pi in group:
                k = kk[pi]
                lim = int(T_kb[k, b]) if k >= SPILL_K else Rb
                cand = [r for r in range(min(Rb, lim))
                        if r not in used and tile_base[k, b, r] >= 0
                        and fill[k, b, r] < 128]
                if not cand:
                    return None
                r = min(cand, key=lambda rr: fill[k, b, rr])
                out[pi] = tile_base[k, b, r] * 128 + fill[k, b, r]
                fill[k, b, r] += 1
                used.add(r)
            i = j
    return out


def _prep(x, W, gamma, beta, in_idx, out_idx):
    k_arr = np.repeat(np.arange(K3, dtype=np.int64), M)
    in_flat = in_idx.reshape(-1).astype(np.int64)
    out_flat = out_idx.reshape(-1).astype(np.int64)
    owner = out_flat // VS

    cores = []
    cnt = np.zeros((NCORES, K3, 4), dtype=np.int64)
    for c in range(NCORES):
        sel = owner == c
        src, tgt, kk = in_flat[sel], out_flat[sel] - c * VS, k_arr[sel]
        b = np.minimum(src // WIN, 3)
        for k in range(K3):
            ks = kk == k
            cnt[c, k] += np.bincount(b[ks], minlength=4)
        cores.append((src, tgt, kk))

    # per-(voxel,bucket) multiplicity determines minimum rounds per bucket
    mult_b = np.zeros(4, dtype=np.int64)
    for c in range(NCORES):
        src, tgt, kk = cores[c]
        b = np.minimum(src // WIN, 3)
        for bb in range(4):
            sel = b == bb
            if sel.sum():
                mult_b[bb] = max(mult_b[bb],
                                 np.bincount(tgt[sel]).max())

    SPILL_K = 8
    slack = 0
    while True:
        T_kb = -(-cnt.max(axis=0) // 128) + slack
        T_kb = np.maximum(T_kb, 1)
        rounds_b = np.maximum(T_kb.max(axis=0), mult_b + slack)
        maxR = int(rounds_b.max())
        tile_base = np.full((K3, 4, maxR), -1, dtype=np.int64)
        calls = []
        ktile = []
        t = 0
        for b in range(4):
            for r in range(int(rounds_b[b])):
                ks = [k for k in range(K3)
                      if T_kb[k, b] > r or (k < SPILL_K)]
                if not ks:
                    continue
                calls.append((b, len(ks), t))
                for k in ks:
                    tile_base[k, b, r] = t
                    ktile.append(k)
                    t += 1
        T_TOT = t
        slots = [_assign(src, tgt, kk, T_kb, rounds_b, tile_base, SPILL_K)
                 for (src, tgt, kk) in cores]
        if all(s is not None for s in slots):
            break
        slack += 1
        assert slack < 6, "round assignment failed repeatedly"

    in_maps = []
    for c in range(NCORES):
        src, tgt, kk = cores[c]
        pos = slots[c]
        gx = np.zeros(T_TOT * 128, dtype=np.int16)
        sx = np.full(T_TOT * 128, TRASH, dtype=np.int16)
        gx[pos] = (src - BASES[np.minimum(src // WIN, 3)]).astype(np.int16)
        sx[pos] = tgt.astype(np.int16)
        wrap = lambda a: np.tile(a.reshape(-1, 16).T.copy(), (8, 1))
        in_maps.append({
            "x": np.ascontiguousarray(x, dtype=np.float32),
            "gx": wrap(gx),
            "sidx": wrap(sx),
            "w": np.ascontiguousarray(np.concatenate(
                [W.transpose(1, 0, 2).reshape(C, K3 * C)] * 2, axis=0),
                dtype=np.float32),
            "gb": np.concatenate([gamma, beta]).astype(np.float32).reshape(1, 2 * C),
            "opart": (np.arange(128) < (VS - (VPAD // 128 - 1) * 128)
                      ).astype(np.float32).reshape(128, 1),
        })
    cfg = {"calls": calls, "ktile": ktile, "T_TOT": T_TOT}
    return in_maps, cfg


def kernel(x, W, gamma, beta, in_idx, out_idx):
    from concourse.bass_utils import run_bass_kernel_spmd

    x = np.asarray(x); W = np.asarray(W)
    gamma = np.asarray(gamma); beta = np.asarray(beta)
    in_idx = np.asarray(in_idx); out_idx = np.asarray(out_idx)

    in_maps, cfg = _prep(x, W, gamma, beta, in_idx, out_idx)
    key = (cfg["T_TOT"], tuple(cfg["ktile"]),
           tuple((b, n, t) for b, n, t in cfg["calls"]))
    if key not in _cache:
        _cache[key] = _build(cfg)
    nc = _cache[key]
    res = run_bass_kernel_spmd(nc, in_maps, core_ids=list(range(NCORES)))
    return np.concatenate([res.results[c]["out"] for c in range(NCORES)], axis=0)
